# revision 19
# baseline (speedup 1.0000x reference)
"""Trainium2 Bass kernel for nn_ApproxExp_FXP32in16out14 (histogram_binning).

Reference semantics: fixed-point piecewise-linear LUT approximation of exp(x)
over 17 uniform breakpoints on [-10, 4] (FXP32.16 in, FXP16.14 out), including
int32-wraparound artifacts of the torch reference in segments 14/15.

The LUT values y0[k] = rint(2^14 exp(-10+0.875k)) are geometric to ~0.35% for
the segments that contain data, and the interpolation weight is affine in x, so
the whole map factors as

    out(x) ~= exp(0.875*k - c0) * ((8/7)*x - k + c1),   k = rne((8/7)*x + 153/14)

The host downcasts x to fp16 before upload (the interpolation factor is only
ever used at fp16 precision, so the extra binning noise is ~0.2% rms), which
halves the input HBM traffic: 1 MiB fp16 in + 1 MiB fp16 out per [128,4096]
tile ~= 6us/tile DMA floor. Engine split, balanced to that floor (measured
rates; GpSimd stays idle -- it shares an exclusively-locked SBUF port pair
with DVE, so gp compute serializes against any 2-input DVE op):
  ScalarE : Ks kq[:, :KS] = rne-i16((8/7)x + 153/14 - 12)  (Copy-act, 2.6us)
            E  ys = exp(.875 kq + b) -> fp16               (Exp-act, 3.6us)
  DVE     : W  w = (8/7)x + (T3_ADD-12) -> fp16            (TS 4x, 1.1us)
            Kd kq[:, KS:]                                  (TS 4x, 0.5us)
            V  v = w - kq -> fp16                          (TT 2x, 2.2us)
            O  out = v * ys -> fp16                        (TT 2x, 2.2us)
Output fp16 (~0.03% rel rms; the gate is 2e-2), upcast on host. A
deterministic ~0.3% of elements (int32-wraparound bands at x>=2.7773, the
x>=4 clamp, deep tail x<-4.7) is recomputed exactly on host.

Sharding: pure data parallel, leading dim 64 -> 8 cores x 8.
"""

import math
import os
from contextlib import ExitStack

import numpy as np

import concourse.bass as bass
import concourse.mybir as mybir
from concourse.bass_utils import run_bass_kernel_spmd

# ---------------------------------------------------------------- constants
FULL_SHAPE = (64, 4096, 1024)
N_CORES = 8
TILES, P, F = 64, 128, 4096  # per-core: 64 tiles of [128, 4096]
NBUF = 4
KSHIFT = 12                  # integer shift folded into kq (keeps w in fp16 range)
KS = 2432                    # quantize cols [0:KS] on ScalarE, [KS:] on DVE

RHO = math.exp(0.875) - 1.0
CONST = 1.0 + RHO / 32768.0          # +0.5 LSB rounding offset of t_fx in Q14
B_SL = RHO / CONST                   # k-coefficient before unit-rescale
CONST1 = 1.0 + (655360.0 / 57344.0) * RHO / CONST
AK_SCALE = 8.0 / 7.0                 # 65536/57344
AK_BIAS = 153.0 / 14.0 - KSHIFT      # quantize bias, with the -12 shift
A2_SCALE = 0.875
A2_BIAS = -10.0 + math.log(CONST) + math.log(B_SL) + A2_SCALE * KSHIFT
T3_ADD = CONST1 / B_SL               # (V0 + T3_ADD) * y2S'
W_BIAS = T3_ADD - KSHIFT             # w = (8/7)x + W_BIAS;  v = w - kq

# host-fixup region boundaries (float32 compares on raw x)
FIX_HI = np.float32(2.7773)          # below first int32-wrap threshold (2.77735)
FIX_LO = np.float32(-4.7)            # deep tail: LUT quantization breaks the model

# ------------------------------------------------------------ bass builder
_NC = None


def _build_nc(tiles: int = TILES) -> bass.Bass:
    f32, i16, fp16 = mybir.dt.float32, mybir.dt.int16, mybir.dt.float16
    A = mybir.AluOpType
    nc = bass.Bass()
    x_ext = nc.declare_dram_parameter("x", [tiles, P, F], fp16, isOutput=False)
    o_ext = nc.declare_dram_parameter("out", [tiles, P, F], fp16, isOutput=True)

    # [128,1] constant for the Exp activation bias (const_aps only has 0/1).
    bias_t = nc.alloc_sbuf_tensor("const-a2bias", [P, 1], f32)
    nc.gpsimd.memset(bias_t.ap(), A2_BIAS)
    nc.all_engine_barrier()
    a2_bias_ap = bias_t.ap()

    ctx = ExitStack()
    xt = [ctx.enter_context(nc.sbuf_tensor(f"xt{j}", [P, F], fp16)) for j in range(NBUF)]
    kq = [ctx.enter_context(nc.sbuf_tensor(f"kq{j}", [P, F], i16)) for j in range(NBUF)]
    wh = [ctx.enter_context(nc.sbuf_tensor(f"wh{j}", [P, F], fp16)) for j in range(NBUF)]
    vh = [ctx.enter_context(nc.sbuf_tensor(f"vh{j}", [P, F], fp16)) for j in range(NBUF)]
    ys = [ctx.enter_context(nc.sbuf_tensor(f"ys{j}", [P, F], fp16)) for j in range(NBUF)]
    ot = [ctx.enter_context(nc.sbuf_tensor(f"ot{j}", [P, F], fp16)) for j in range(NBUF)]
    # per-buffer-slot DMA semaphores: at most one in-flight DMA per sem, so a
    # waiter on >=16*n can't be satisfied by interleaved partial completions
    # of two DMAs (the 16 per-engine increments of concurrent DMAs interleave).
    s_in = [ctx.enter_context(nc.semaphore(f"s_in{j}")) for j in range(NBUF)]
    s_out = [ctx.enter_context(nc.semaphore(f"s_out{j}")) for j in range(NBUF)]
    s_k = ctx.enter_context(nc.semaphore("s_k"))    # ScalarE K cols done
    s_kd = ctx.enter_context(nc.semaphore("s_kd"))  # DVE K cols done
    s_y = ctx.enter_context(nc.semaphore("s_y"))
    s_w0 = ctx.enter_context(nc.semaphore("s_w0"))
    s_v1 = ctx.enter_context(nc.semaphore("s_v1"))
    s_o = ctx.enter_context(nc.semaphore("s_o"))
    block = ctx.enter_context(nc.Block())

    LOOK = NBUF - 1  # input prefetch distance

    @block.sync
    def _(sync):
        for i in range(min(LOOK, tiles)):
            sync.dma_start(out=xt[i % NBUF][:], in_=x_ext[i]).then_inc(
                s_in[i % NBUF], 16
            )
        for i in range(tiles):
            # out(i) first: the in-stream has ~LOOK tiles of slack, so the
            # s_o wait here does not starve the input side.
            sync.wait_ge(s_o, i + 1)
            sync.dma_start(out=o_ext[i], in_=ot[i % NBUF][:]).then_inc(
                s_out[i % NBUF], 16
            )
            if i + LOOK < tiles:
                # xt[(i+LOOK)%NBUF] is read by Ks/Kd/W of tile i-1 only
                if i >= 1:
                    sync.wait_ge(s_k, i)   # Ks(i-1) done
                    sync.wait_ge(s_v1, i)  # V(i-1) done => W,Kd(i-1) done
                sync.dma_start(
                    out=xt[(i + LOOK) % NBUF][:], in_=x_ext[i + LOOK]
                ).then_inc(s_in[(i + LOOK) % NBUF], 16)

    @block.scalar
    def _(scalar):
        for i in range(tiles):
            j = i % NBUF
            scalar.wait_ge(s_in[j], 16 * (i // NBUF + 1))
            if i >= NBUF:
                scalar.wait_ge(s_v1, i - NBUF + 1)  # kq slot free (V(i-NBUF))
            # Ks: quantize cols [0:KS]
            nc.scalar.activation(
                kq[j][:, :KS], xt[j][:, :KS], mybir.ActivationFunctionType.Copy,
                bias=AK_BIAS, scale=AK_SCALE,
            ).then_inc(s_k, 1)
            if i >= NBUF:
                scalar.wait_ge(s_o, i - NBUF + 1)  # ys slot free (O(i-NBUF))
            scalar.wait_ge(s_k, i + 1)   # own Ks(i) retired (race-detector sync)
            scalar.wait_ge(s_kd, i + 1)  # DVE Kd(i) done (kq cols [KS:])
            nc.scalar.activation(
                ys[j][:], kq[j][:], mybir.ActivationFunctionType.Exp,
                bias=a2_bias_ap, scale=A2_SCALE,
            ).then_inc(s_y, 1)

    @block.vector
    def _(vector):
        def stage_wk(i):
            """W(i) + Kd(i): the tile's independent TS ops (fp16 4x mode)."""
            j = i % NBUF
            vector.wait_ge(s_in[j], 16 * (i // NBUF + 1))
            if i >= NBUF:
                vector.wait_ge(s_v1, i - NBUF + 1)  # wh slot free (V(i-NBUF))
            nc.vector.tensor_scalar(
                out=wh[j][:], in0=xt[j][:], scalar1=AK_SCALE, scalar2=W_BIAS,
                op0=A.mult, op1=A.add,
            ).then_inc(s_w0, 1)
            if i >= NBUF:
                vector.wait_ge(s_y, i - NBUF + 1)  # kq[KS:] slot: E(i-NBUF) read
            nc.vector.tensor_scalar(
                out=kq[j][:, KS:], in0=xt[j][:, KS:], scalar1=AK_SCALE,
                scalar2=AK_BIAS, op0=A.mult, op1=A.add,
            ).then_inc(s_kd, 1)

        stage_wk(0)
        for i in range(tiles):
            j = i % NBUF
            vector.wait_ge(s_k, i + 1)   # ScalarE Ks(i) done
            vector.wait_ge(s_w0, i + 1)  # own W(i) retired (race-detector sync)
            vector.wait_ge(s_kd, i + 1)  # own Kd(i) retired (race-detector sync)
            if i >= NBUF:
                vector.wait_ge(s_o, i - NBUF + 1)  # vh slot free (O(i-NBUF))
            # V: v = w - kq   (fp16 - i16 -> fp16, TT 2x)
            nc.vector.tensor_tensor(
                out=vh[j][:], in0=wh[j][:], in1=kq[j][:], op=A.subtract,
            ).then_inc(s_v1, 1)
            # issue next tile's W/Kd before O(i): O waits on E(i), and E(i+1)
            # needs Kd(i+1) -- this keeps that off the cross-engine cycle.
            if i + 1 < tiles:
                stage_wk(i + 1)
            vector.wait_ge(s_y, i + 1)
            vector.wait_ge(s_v1, i + 1)  # own V(i) retired (race-detector sync)
            if i >= NBUF:
                vector.wait_ge(s_out[j], 16 * (i // NBUF))  # ot slot free
            # O: out = v * ys   (fp16 TT 2x)
            nc.vector.tensor_tensor(
                out=ot[j][:], in0=vh[j][:], in1=ys[j][:], op=A.mult,
            ).then_inc(s_o, 1)

    ctx.close()
    return nc


def _get_nc() -> bass.Bass:
    global _NC
    if _NC is None:
        _NC = _build_nc()
    return _NC


# ------------------------------------------------- exact host-side reference
_XP = np.round(np.linspace(-10.0, 4.0, 17) * 65536.0).astype(np.int64)
_YV = np.round(np.exp(np.linspace(-10.0, 4.0, 17)) * 16384.0).astype(np.int64)
_DY = np.diff(_YV)


def _reference_exact(xs: np.ndarray) -> np.ndarray:
    """Bit-faithful int32 reference for a (small) subset of elements."""
    x_int = np.rint(xs.astype(np.float64) * 65536.0).astype(np.int64)
    mask_low = x_int <= _XP[0]
    mask_high = x_int >= _XP[-1]
    xc = np.clip(x_int, _XP[0], _XP[-1])
    idx = np.clip(np.searchsorted(_XP, xc, side="left") - 1, 0, 15)
    dxv = xc - _XP[idx]
    t_fx = ((dxv << 14) + 28672) // 57344
    prod = t_fx * _DY[idx] + 8192
    pm = prod & 0xFFFFFFFF
    S = np.where(pm >= 1 << 31, pm - (1 << 32), pm)
    interp = _YV[idx] + (S >> 14)
    out_int = np.where(mask_low, _YV[0], np.where(mask_high, _YV[-1], interp))
    return (out_int.astype(np.float32) / np.float32(16384.0)).astype(np.float32)


def _host_fixup(x_flat: np.ndarray, out_flat: np.ndarray) -> None:
    sel = (x_flat >= FIX_HI) | (x_flat < FIX_LO)
    idxs = np.flatnonzero(sel)
    if idxs.size:
        out_flat[idxs] = _reference_exact(x_flat[idxs])


_last_results = None


def kernel(x: np.ndarray) -> np.ndarray:
    assert x.shape == FULL_SHAPE and x.dtype == np.float32, (x.shape, x.dtype)
    nc = _get_nc()
    per = FULL_SHAPE[0] // N_CORES
    x16 = x.astype(np.float16)
    in_maps = [
        {"x": np.ascontiguousarray(x16[i * per : (i + 1) * per]).reshape(TILES, P, F)}
        for i in range(N_CORES)
    ]
    global _last_results
    res = run_bass_kernel_spmd(nc, in_maps, core_ids=list(range(N_CORES)))
    _last_results = res
    out = np.concatenate(
        [
            r["out"].astype(np.float32).reshape(per, FULL_SHAPE[1], FULL_SHAPE[2])
            for r in res.results
        ],
        axis=0,
    )
    _host_fixup(x.ravel(), out.ravel())
    return out
